# revision 1
# baseline (speedup 1.0000x reference)
"""DiSco weighted loss kernel for 8 trn2 NeuronCores.

Math: for symmetric a_ij=|x_i-x_j|, the weighted distance-correlation terms
collapse to  dcov = S_ab - 2*T1ab + g_a*g_b  with
  ar_i = sum_j w_j a_ij,  g_a = sum_i w_i ar_i,  T1ab = sum_i w_i ar_i br_i,
  S_ab = sum_ij w_i w_j a_ij b_ij,
and dvar_x = 2(q - m^2) - 2*T1aa + g_a^2 exactly (|.|^2 loses the abs).
Each core owns 512 i-rows (free axis) and scans all j (partition axis,
32 column-sets of its [128,32] f-major tiles); TensorE reduces over j via
bf16 matmuls accumulated in PSUM; the per-core scalar partials are summed
on the host (8x22 floats) to avoid a ~20us AllReduce latency floor.
"""

from contextlib import ExitStack

import numpy as np
import ml_dtypes

import concourse.bass as bass
from concourse import mybir
from concourse.bass_utils import run_bass_kernel_spmd

F32 = mybir.dt.float32
BF16 = mybir.dt.bfloat16
I32 = mybir.dt.int32
U16 = mybir.dt.uint16
AO = mybir.AluOpType
AF = mybir.ActivationFunctionType
AX = mybir.AxisListType

N, C, NCORES = 4096, 3, 8
M = N // NCORES  # 512 rows per core
NB = 32  # j-sets (columns of the [128,32] tiles)

DISCO_LAMBDA = 0.1
EPS_W = 1e-8
EPS_VAR = 1e-10


def _build_program():
    nc = bass.Bass()
    lg = nc.declare_dram_parameter("lg", [N, C], F32, isOutput=False)
    lgm = nc.declare_dram_parameter("lgm", [M, C], F32, isOutput=False)
    tg = nc.declare_dram_parameter("tg", [N], I32, isOutput=False)
    wf = nc.declare_dram_parameter("wf", [N], F32, isOutput=False)
    wm = nc.declare_dram_parameter("wm", [1, M], F32, isOutput=False)
    m1f = nc.declare_dram_parameter("m1f", [N], BF16, isOutput=False)
    m2f = nc.declare_dram_parameter("m2f", [N], BF16, isOutput=False)
    m1m = nc.declare_dram_parameter("m1m", [1, M], BF16, isOutput=False)
    m2m = nc.declare_dram_parameter("m2m", [1, M], BF16, isOutput=False)
    out = nc.declare_dram_parameter("out", [32], F32, isOutput=True)
    scr = nc.dram_tensor("scr", [1, M], BF16)

    es = ExitStack()
    def sb(name, shp, dt):
        return es.enter_context(nc.sbuf_tensor(name, shp, dt))

    def ps(name, shp):
        return es.enter_context(nc.psum_tensor(name, shp, F32))

    lgt = sb("lgt", [128, 96], F32)
    lgtm = sb("lgtm", [128, 12], F32)
    tgt = sb("tgt", [128, 32], I32)
    wt = sb("wt", [128, 32], F32)
    m1t = sb("m1t", [128, 32], BF16)
    m2t = sb("m2t", [128, 32], BF16)
    wrow = sb("wrow", [2, M], F32)
    mrow = sb("mrow", [2, M], BF16)
    y1row = sb("y1row", [128, M], BF16)
    y2row = sb("y2row", [128, M], BF16)
    xrow = sb("xrow", [128, M], BF16)

    e = sb("e", [128, 96], F32)
    den = sb("den", [128, 32], F32)
    rec = sb("rec", [128, 32], F32)
    sc = sb("sc", [128, 32], F32)
    scbf = sb("scbf", [128, 32], BF16)
    sc_r = sb("sc_r", [128, 32], F32)  # bf16-rounded scores back in f32
    em = sb("em", [128, 12], F32)
    denm = sb("denm", [128, 4], F32)
    recm = sb("recm", [128, 4], F32)
    scm = sb("scm", [128, 4], F32)
    scmbf = sb("scmbf", [128, 4], BF16)
    ny1 = sb("ny1", [128, 32], F32)
    ny2 = sb("ny2", [128, 32], F32)
    msk1 = sb("msk1", [128, 32], F32)
    msk2 = sb("msk2", [128, 32], F32)
    w1 = sb("w1", [128, 32], F32)
    w2 = sb("w2", [128, 32], F32)
    w12 = sb("w12", [128, 64], BF16)
    m1ff = sb("m1ff", [128, 32], F32)
    m2ff = sb("m2ff", [128, 32], F32)
    sq = sb("sq", [128, 32], F32)  # scratch squares/products
    pr = sb("pr", [128, 32], F32)
    tgtf = sb("tgtf", [128, 32], F32)
    sel = sb("sel", [128, 32], F32)
    lt = sb("lt", [128, 32], F32)
    lnden = sb("lnden", [128, 32], F32)
    ce = sb("ce", [128, 32], F32)
    G = sb("G", [128, 12], F32)
    Gsb = sb("Gsb", [1, 12], F32)
    ones_t = sb("ones_t", [128, 1], F32)

    # double-buffered loop tiles
    d_ = [sb(f"d{i}", [128, 2 * M], BF16) for i in range(3)]
    a_ = [sb(f"a{i}", [128, 2 * M], BF16) for i in range(3)]
    b1_ = [sb(f"b1{i}", [128, 2 * M], BF16) for i in range(3)]
    b2_ = [sb(f"b2{i}", [128, 2 * M], BF16) for i in range(3)]
    ab1_ = [sb(f"ab1{i}", [128, 2 * M], BF16) for i in range(3)]
    ab2_ = [sb(f"ab2{i}", [128, 2 * M], BF16) for i in range(3)]

    mrow_w = sb("mrow_w", [2, M], F32)
    w12row = sb("w12row", [2, M], F32)
    arsb = sb("arsb", [2, M], F32)
    V = sb("V", [2, 2 * M], F32)
    t_ = sb("t_", [2, M], F32)
    t2_ = sb("t2_", [2, M], F32)
    sc2 = sb("sc2", [2, 8], F32)
    st_b2 = sb("st_b2", [2, M], F32)
    st_p2 = sb("st_p2", [2, M], F32)
    w12row_bf = sb("w12row_bf", [2, M], BF16)

    ps_a = ps("ps_a", [2, M])
    ps_b1 = ps("ps_b1", [2, M])
    ps_b2 = ps("ps_b2", [2, M])
    ps_p1 = ps("ps_p1", [2, M])
    ps_p2 = ps("ps_p2", [2, M])
    ps_g = ps("ps_g", [1, 12])

    dm = es.enter_context(nc.semaphore("dm"))
    sa = es.enter_context(nc.semaphore("sa"))  # ACT progress
    svp = es.enter_context(nc.semaphore("svp"))  # DVE phase-0 progress
    sv = es.enter_context(nc.semaphore("sv"))  # DVE loop progress
    sp = es.enter_context(nc.semaphore("sp"))  # PE progress
    sa2 = es.enter_context(nc.semaphore("sa2"))  # ACT phase-2 copies
    block = es.enter_context(nc.Block())

    @block.sync
    def _(sync):
        # 10 input DMAs (dm: 16 each -> 160 when all in)
        sync.dma_start(out=lgt[:], in_=lg[:].rearrange("(p f) c -> p (f c)", f=32)).then_inc(dm, 16)
        sync.dma_start(out=lgtm[:], in_=lgm[:].rearrange("(p q) c -> p (q c)", q=4)).then_inc(dm, 16)
        sync.dma_start(out=tgt[:], in_=tg[:].rearrange("(p f) -> p f", f=32)).then_inc(dm, 16)
        sync.dma_start(out=wt[:], in_=wf[:].rearrange("(p f) -> p f", f=32)).then_inc(dm, 16)
        sync.dma_start(out=m1t[:], in_=m1f[:].rearrange("(p f) -> p f", f=32)).then_inc(dm, 16)
        sync.dma_start(out=m2t[:], in_=m2f[:].rearrange("(p f) -> p f", f=32)).then_inc(dm, 16)
        sync.dma_start(out=wrow[:], in_=wm[:].broadcast_to([2, M])).then_inc(dm, 16)
        sync.dma_start(out=mrow[0:1, :], in_=m1m[:]).then_inc(dm, 16)
        sync.dma_start(out=mrow[1:2, :], in_=m2m[:]).then_inc(dm, 16)
        sync.dma_start(out=y1row[:], in_=m1m[:].broadcast_to([128, M])).then_inc(dm, 16)
        sync.dma_start(out=y2row[:], in_=m2m[:].broadcast_to([128, M])).then_inc(dm, 16)
        # scores-mine roundtrip: wait for scmbf (svp>=1)
        sync.wait_ge(svp, 1)
        sync.dma_start(out=scr[:].rearrange("a b -> (a b)"), in_=scmbf[:]).then_inc(dm, 16)
        sync.wait_ge(dm, 16 * 12)
        sync.dma_start(out=xrow[:], in_=scr[:].broadcast_to([128, M])).then_inc(dm, 16)
        # phase-2 row moves (partition 1 via DMA)
        sync.wait_ge(sa2, 4)
        sync.dma_start(out=V[1:2, 0:M], in_=st_b2[1:2, :]).then_inc(dm, 16)
        sync.dma_start(out=V[1:2, M : 2 * M], in_=st_p2[1:2, :]).then_inc(dm, 16)
        # outputs
        sync.wait_ge(sv, 200)
        sync.dma_start(out=out[0:12], in_=Gsb[:]).then_inc(dm, 16)
        sync.dma_start(out=out[12:28], in_=sc2[:]).then_inc(dm, 16)

    @block.scalar
    def _(scalar):
        scalar.wait_ge(dm, 16 * 11)  # all initial loads in
        scalar.activation(e[:], lgt[:], AF.Exp).then_inc(sa, 1)
        scalar.activation(em[:], lgtm[:], AF.Exp).then_inc(sa, 1)
        scalar.wait_ge(svp, 2)  # den ready
        scalar.activation(lnden[:], den[:], AF.Ln).then_inc(sa, 1)
        # loop: b1/b2 per j-set, double buffered
        scalar.wait_ge(svp, 3)  # ny1, ny2 ready
        for K in range(NB // 2):
            s = K % 3
            if K >= 3:
                scalar.wait_ge(sp, K - 1)
            k0, k1 = 2 * K, 2 * K + 1
            scalar.activation(b1_[s][:, 0:M], y1row[:], AF.Abs, bias=ny1[:, k0 : k0 + 1]).then_inc(sa, 1)
            scalar.activation(b1_[s][:, M : 2 * M], y1row[:], AF.Abs, bias=ny1[:, k1 : k1 + 1]).then_inc(sa, 1)
            scalar.activation(b2_[s][:, 0:M], y2row[:], AF.Abs, bias=ny2[:, k0 : k0 + 1]).then_inc(sa, 1)
            scalar.activation(b2_[s][:, M : 2 * M], y2row[:], AF.Abs, bias=ny2[:, k1 : k1 + 1]).then_inc(sa, 1)
        # phase 2 copies out of PSUM
        scalar.wait_ge(sp, NB // 2 + 1)
        scalar.activation(V[0:1, 0:M], ps_b1[0:1, :], AF.Copy).then_inc(sa2, 1)
        scalar.activation(V[0:1, M : 2 * M], ps_p1[0:1, :], AF.Copy).then_inc(sa2, 1)
        scalar.activation(st_b2[:], ps_b2[:], AF.Copy).then_inc(sa2, 1)
        scalar.activation(st_p2[:], ps_p2[:], AF.Copy).then_inc(sa2, 1)

    @block.vector
    def _(vector):
        # ---- phase 0 ----
        vector.memset(ones_t[:], 1.0)
        vector.drain()
        vector.wait_ge(sa, 2)
        vector.tensor_reduce(denm[:], em[:].rearrange("p (f c) -> p f c", c=3), AX.X, AO.add)
        vector.drain()
        vector.reciprocal(recm[:], denm[:])
        vector.drain()
        vector.tensor_tensor(scm[:], em[:, 0:12:3], recm[:], AO.mult)
        vector.drain()
        vector.tensor_copy(scmbf[:], scm[:])
        vector.drain().then_inc(svp, 1)
        vector.tensor_reduce(den[:], e[:].rearrange("p (f c) -> p f c", c=3), AX.X, AO.add)
        vector.drain()
        vector.reciprocal(rec[:], den[:])
        vector.drain()
        vector.tensor_tensor(sc[:], e[:, 0:96:3], rec[:], AO.mult)
        vector.drain()
        vector.tensor_copy(scbf[:], sc[:])
        vector.drain()
        vector.tensor_copy(sc_r[:], scbf[:])
        vector.drain().then_inc(svp, 1)
        vector.wait_ge(dm, 16 * 11)  # all initial loads in
        vector.tensor_scalar(ny1[:], m1t[:], -1.0, None, AO.mult)
        vector.drain()
        vector.tensor_scalar(ny2[:], m2t[:], -1.0, None, AO.mult)
        vector.drain().then_inc(svp, 1)
        vector.tensor_scalar(msk1[:], m1t[:], 0.0, None, AO.is_gt)
        vector.drain()
        vector.tensor_scalar(msk2[:], m2t[:], 0.0, None, AO.is_gt)
        vector.drain()
        vector.tensor_tensor(w1[:], wt[:], msk1[:], AO.mult)
        vector.drain()
        vector.tensor_tensor(w2[:], wt[:], msk2[:], AO.mult)
        vector.drain()
        vector.tensor_copy(w12[:, 0:64:2], w1[:])
        vector.drain()
        vector.tensor_copy(w12[:, 1:64:2], w2[:])
        vector.drain().then_inc(svp, 1)
        vector.tensor_copy(w1[:], w12[:, 0:64:2])
        vector.drain()
        vector.tensor_copy(w2[:], w12[:, 1:64:2])
        vector.drain()
        # CE: lt = logits[target]
        vector.tensor_copy(tgtf[:], tgt[:])
        vector.drain()
        vector.tensor_scalar(sel[:], tgtf[:], 0.0, None, AO.is_equal)
        vector.drain()
        vector.tensor_tensor(lt[:], lgt[:, 0:96:3], sel[:], AO.mult)
        vector.drain()
        vector.tensor_scalar(sel[:], tgtf[:], 1.0, None, AO.is_equal)
        vector.drain()
        vector.tensor_tensor(pr[:], lgt[:, 1:96:3], sel[:], AO.mult)
        vector.drain()
        vector.tensor_tensor(lt[:], lt[:], pr[:], AO.add)
        vector.drain()
        vector.tensor_scalar(sel[:], tgtf[:], 2.0, None, AO.is_equal)
        vector.drain()
        vector.tensor_tensor(pr[:], lgt[:, 2:96:3], sel[:], AO.mult)
        vector.drain()
        vector.tensor_tensor(lt[:], lt[:], pr[:], AO.add)
        vector.drain()
        vector.wait_ge(sa, 3)  # lnden
        vector.tensor_tensor(ce[:], lnden[:], lt[:], AO.subtract)
        vector.drain()
        vector.tensor_tensor(pr[:], wt[:], ce[:], AO.mult)
        vector.drain()
        # G columns: Sw S1 S2 CE m1 q1 my1 qy1 m2 q2 my2 qy2
        vector.tensor_reduce(G[:, 0:1], wt[:], AX.X, AO.add)
        vector.drain()
        vector.tensor_reduce(G[:, 1:2], w1[:], AX.X, AO.add)
        vector.drain()
        vector.tensor_reduce(G[:, 2:3], w2[:], AX.X, AO.add)
        vector.drain()
        vector.tensor_reduce(G[:, 3:4], pr[:], AX.X, AO.add)
        vector.drain()
        vector.tensor_copy(m1ff[:], m1t[:])
        vector.drain()
        vector.tensor_copy(m2ff[:], m2t[:])
        vector.drain()
        vector.tensor_tensor(pr[:], w1[:], sc_r[:], AO.mult)
        vector.drain()
        vector.tensor_reduce(G[:, 4:5], pr[:], AX.X, AO.add)
        vector.drain()
        vector.tensor_tensor(sq[:], sc_r[:], sc_r[:], AO.mult)
        vector.drain()
        vector.tensor_tensor(pr[:], w1[:], sq[:], AO.mult)
        vector.drain()
        vector.tensor_reduce(G[:, 5:6], pr[:], AX.X, AO.add)
        vector.drain()
        vector.tensor_tensor(pr[:], w1[:], m1ff[:], AO.mult)
        vector.drain()
        vector.tensor_reduce(G[:, 6:7], pr[:], AX.X, AO.add)
        vector.drain()
        vector.tensor_tensor(pr[:], m1ff[:], m1ff[:], AO.mult)
        vector.drain()
        vector.tensor_tensor(pr[:], w1[:], pr[:], AO.mult)
        vector.drain()
        vector.tensor_reduce(G[:, 7:8], pr[:], AX.X, AO.add)
        vector.drain()
        vector.tensor_tensor(pr[:], w2[:], sc_r[:], AO.mult)
        vector.drain()
        vector.tensor_reduce(G[:, 8:9], pr[:], AX.X, AO.add)
        vector.drain()
        vector.tensor_tensor(pr[:], w2[:], sq[:], AO.mult)
        vector.drain()
        vector.tensor_reduce(G[:, 9:10], pr[:], AX.X, AO.add)
        vector.drain()
        vector.tensor_tensor(pr[:], w2[:], m2ff[:], AO.mult)
        vector.drain()
        vector.tensor_reduce(G[:, 10:11], pr[:], AX.X, AO.add)
        vector.drain()
        vector.tensor_tensor(pr[:], m2ff[:], m2ff[:], AO.mult)
        vector.drain()
        vector.tensor_tensor(pr[:], w2[:], pr[:], AO.mult)
        vector.drain()
        vector.tensor_reduce(G[:, 11:12], pr[:], AX.X, AO.add)
        vector.drain().then_inc(svp, 1)
        # w12row for phase 2
        vector.tensor_scalar(mrow_w[:], mrow[:], 0.0, None, AO.is_gt)
        vector.drain()
        vector.tensor_tensor(w12row[:], wrow[:], mrow_w[:], AO.mult)
        vector.drain()
        vector.tensor_copy(w12row_bf[:], w12row[:])
        vector.drain()
        vector.tensor_copy(w12row[:], w12row_bf[:])
        vector.drain()
        # ---- phase 1 loop ----
        vector.wait_ge(dm, 16 * 13)  # xrow in
        for K in range(NB // 2):
            s = K % 3
            if K >= 3:
                vector.wait_ge(sp, K - 1)
            k0, k1 = 2 * K, 2 * K + 1
            vector.tensor_scalar(d_[s][:, 0:M], xrow[:], sc_r[:, k0 : k0 + 1], None, AO.subtract)
            vector.tensor_scalar(d_[s][:, M : 2 * M], xrow[:], sc_r[:, k1 : k1 + 1], None, AO.subtract)
            vector.drain()
            vector.tensor_scalar(
                a_[s][:].bitcast(U16), d_[s][:].bitcast(U16), 0x7FFF, None, AO.bitwise_and
            )
            vector.drain().then_inc(sv, 1)
            vector.wait_ge(sa, 3 + 4 * K + 2)
            vector.tensor_tensor(ab1_[s][:], a_[s][:], b1_[s][:], AO.mult)
            vector.wait_ge(sa, 3 + 4 * K + 4)
            vector.tensor_tensor(ab2_[s][:], a_[s][:], b2_[s][:], AO.mult)
            vector.drain().then_inc(sv, 1)
        # ---- phase 2 ----
        vector.wait_ge(sp, NB // 2 + 1)
        vector.tensor_copy(arsb[:], ps_a[:])
        vector.drain()
        vector.tensor_tensor(t_[:], arsb[:], w12row[:], AO.mult)
        vector.drain()
        vector.tensor_reduce(sc2[:, 0:1], t_[:], AX.X, AO.add)  # g_a
        vector.drain()
        vector.tensor_tensor(t2_[:], t_[:], arsb[:], AO.mult)
        vector.drain()
        vector.tensor_reduce(sc2[:, 1:2], t2_[:], AX.X, AO.add)  # T1aa
        vector.drain()
        vector.wait_ge(sa2, 4)
        vector.wait_ge(dm, 16 * 15)
        vector.tensor_tensor(t2_[:], V[:, 0:M], w12row[:], AO.mult)
        vector.drain()
        vector.tensor_reduce(sc2[:, 2:3], t2_[:], AX.X, AO.add)  # g_b
        vector.drain()
        vector.tensor_tensor(t2_[:], t2_[:], V[:, 0:M], AO.mult)
        vector.drain()
        vector.tensor_reduce(sc2[:, 3:4], t2_[:], AX.X, AO.add)  # T1bb
        vector.drain()
        vector.tensor_tensor(t2_[:], t_[:], V[:, 0:M], AO.mult)
        vector.drain()
        vector.tensor_reduce(sc2[:, 4:5], t2_[:], AX.X, AO.add)  # T1ab
        vector.drain()
        vector.tensor_tensor(t2_[:], V[:, M : 2 * M], w12row[:], AO.mult)
        vector.drain()
        vector.tensor_reduce(sc2[:, 5:6], t2_[:], AX.X, AO.add)  # S_ab
        vector.drain()
        vector.tensor_copy(Gsb[:], ps_g[:])
        vector.drain().then_inc(sv, 200)

    @block.tensor
    def _(tensor):
        tensor.wait_ge(svp, 5)  # G ready
        tensor.matmul(ps_g[:], ones_t[:], G[:], start=True, stop=True).then_inc(sp, 1)
        for K in range(NB // 2):
            s = K % 3
            st = K == 0
            last = K == NB // 2 - 1
            k0, k1 = 2 * K, 2 * K + 1
            lw0 = w12[:, 2 * k0 : 2 * k0 + 2]
            lw1 = w12[:, 2 * k1 : 2 * k1 + 2]
            tensor.wait_ge(sa, 3 + 4 * K + 4)
            tensor.matmul(ps_b1[:], lw0, b1_[s][:, 0:M], start=st, stop=False)
            tensor.matmul(ps_b1[:], lw1, b1_[s][:, M : 2 * M], start=False, stop=last)
            tensor.matmul(ps_b2[:], lw0, b2_[s][:, 0:M], start=st, stop=False)
            tensor.matmul(ps_b2[:], lw1, b2_[s][:, M : 2 * M], start=False, stop=last)
            tensor.wait_ge(sv, 2 * K + 2)
            tensor.matmul(ps_a[:], lw0, a_[s][:, 0:M], start=st, stop=False)
            tensor.matmul(ps_a[:], lw1, a_[s][:, M : 2 * M], start=False, stop=last)
            tensor.matmul(ps_p1[:], lw0, ab1_[s][:, 0:M], start=st, stop=False)
            tensor.matmul(ps_p1[:], lw1, ab1_[s][:, M : 2 * M], start=False, stop=last)
            tensor.matmul(ps_p2[:], lw0, ab2_[s][:, 0:M], start=st, stop=False)
            tensor.matmul(ps_p2[:], lw1, ab2_[s][:, M : 2 * M], start=False, stop=last).then_inc(sp, 1)

    return nc, es


_NC_CACHE = {}


def kernel(logits, target, weight, mass1, mass2):
    logits = np.asarray(logits, dtype=np.float32)
    target_i = np.asarray(target).astype(np.int32)
    weight = np.asarray(weight, dtype=np.float32)
    mass1 = np.asarray(mass1, dtype=np.float32)
    mass2 = np.asarray(mass2, dtype=np.float32)
    m1b = mass1.astype(ml_dtypes.bfloat16)
    m2b = mass2.astype(ml_dtypes.bfloat16)

    if "nc" not in _NC_CACHE:
        _NC_CACHE["nc"] = _build_program()
    nc, _ = _NC_CACHE["nc"]

    in_maps = []
    for c in range(NCORES):
        sl = slice(c * M, (c + 1) * M)
        in_maps.append(
            {
                "lg": logits,
                "lgm": np.ascontiguousarray(logits[sl]),
                "tg": target_i,
                "wf": weight,
                "wm": weight[sl].reshape(1, M),
                "m1f": m1b,
                "m2f": m2b,
                "m1m": m1b[sl].reshape(1, M),
                "m2m": m2b[sl].reshape(1, M),
            }
        )
    res = run_bass_kernel_spmd(nc, in_maps, list(range(NCORES)))
    outs = [r["out"] for r in res.results]
    return _combine(outs)


def _combine(outs):
    G = np.asarray(outs[0][0:12], dtype=np.float64)
    Sw, S1, S2, CEs = G[0], G[1], G[2], G[3]
    m1, q1, my1, qy1 = G[4], G[5], G[6], G[7]
    m2, q2, my2, qy2 = G[8], G[9], G[10], G[11]
    # per-core partials: sc2 [2,8] flattened at out[12:28]
    P = np.zeros((2, 8), dtype=np.float64)
    for o in outs:
        P += o[12:28].reshape(2, 8).astype(np.float64)
    ce_mean = CEs / max(Sw, EPS_W)

    def disco(row, Sr, m, q, my, qy):
        g_a, T1aa, g_b, T1bb, T1ab, S_ab = P[row, 0:6]
        s = 1.0 / max(Sr, EPS_W)
        dcov = s * s * S_ab - 2.0 * s**3 * T1ab + s**4 * g_a * g_b
        dvx = 2.0 * (s * q - (s * m) ** 2) - 2.0 * s**3 * T1aa + (s * s * g_a) ** 2
        dvy = 2.0 * (s * qy - (s * my) ** 2) - 2.0 * s**3 * T1bb + (s * s * g_b) ** 2
        ok = (dvx > EPS_VAR) and (dvy > EPS_VAR)
        if not ok:
            return 0.0
        return np.sqrt(np.abs(dcov) / np.sqrt(dvx * dvy))

    d1 = disco(0, S1, m1, q1, my1, qy1)
    d2 = disco(1, S2, m2, q2, my2, qy2)
    return np.float32(ce_mean + DISCO_LAMBDA * (d1 + d2))



# revision 2
# speedup vs baseline: 4.0598x; 4.0598x over previous
"""DiSco weighted loss kernel for 8 trn2 NeuronCores.

Math: for symmetric a_ij=|x_i-x_j|, the weighted distance-correlation terms
collapse to  dcov = S_ab - 2*T1ab + g_a*g_b  with
  ar_i = sum_j w_j a_ij,  g_a = sum_i w_i ar_i,  T1ab = sum_i w_i ar_i br_i,
  S_ab = sum_ij w_i w_j a_ij b_ij,
and dvar_x = 2(q - m^2) - 2*T1aa + g_a^2 exactly (|.|^2 loses the abs).
Each core owns 512 i-rows (free axis) and scans all j (partition axis,
32 column-sets of its [128,32] f-major tiles); TensorE reduces over j via
bf16 matmuls accumulated in PSUM; the per-core scalar partials are summed
on the host (8x22 floats) to avoid a ~20us AllReduce latency floor.
"""

from contextlib import ExitStack

import numpy as np
import ml_dtypes

import concourse.bass as bass
from concourse import mybir
from concourse.bass_utils import run_bass_kernel_spmd

F32 = mybir.dt.float32
BF16 = mybir.dt.bfloat16
I32 = mybir.dt.int32
U16 = mybir.dt.uint16
AO = mybir.AluOpType
AF = mybir.ActivationFunctionType
AX = mybir.AxisListType

N, C, NCORES = 4096, 3, 8
M = N // NCORES  # 512 rows per core
NB = 32  # j-sets (columns of the [128,32] tiles)

DISCO_LAMBDA = 0.1
EPS_W = 1e-8
EPS_VAR = 1e-10


def _build_program():
    nc = bass.Bass()
    lg = nc.declare_dram_parameter("lg", [N, C], F32, isOutput=False)
    lgm = nc.declare_dram_parameter("lgm", [M, C], F32, isOutput=False)
    tg = nc.declare_dram_parameter("tg", [N], I32, isOutput=False)
    wf = nc.declare_dram_parameter("wf", [N], F32, isOutput=False)
    wm = nc.declare_dram_parameter("wm", [1, M], F32, isOutput=False)
    m1f = nc.declare_dram_parameter("m1f", [N], BF16, isOutput=False)
    m2f = nc.declare_dram_parameter("m2f", [N], BF16, isOutput=False)
    m1m = nc.declare_dram_parameter("m1m", [1, M], BF16, isOutput=False)
    m2m = nc.declare_dram_parameter("m2m", [1, M], BF16, isOutput=False)
    out = nc.declare_dram_parameter("out", [32], F32, isOutput=True)
    scr = nc.dram_tensor("scr", [1, M], BF16)

    es = ExitStack()
    def sb(name, shp, dt):
        return es.enter_context(nc.sbuf_tensor(name, shp, dt))

    def ps(name, shp):
        return es.enter_context(nc.psum_tensor(name, shp, F32))

    lgt = sb("lgt", [128, 96], F32)
    lgtm = sb("lgtm", [128, 12], F32)
    tgt = sb("tgt", [128, 32], I32)
    wt = sb("wt", [128, 32], F32)
    m1t = sb("m1t", [128, 32], BF16)
    m2t = sb("m2t", [128, 32], BF16)
    wrow = sb("wrow", [2, M], F32)
    mrow = sb("mrow", [2, M], BF16)
    y1row = sb("y1row", [128, M], BF16)
    y2row = sb("y2row", [128, M], BF16)
    xrow = sb("xrow", [128, M], BF16)

    e = sb("e", [128, 96], F32)
    den = sb("den", [128, 32], F32)
    rec = sb("rec", [128, 32], F32)
    sc = sb("sc", [128, 32], F32)
    scbf = sb("scbf", [128, 32], BF16)
    sc_r = sb("sc_r", [128, 32], F32)  # bf16-rounded scores back in f32
    em = sb("em", [128, 12], F32)
    denm = sb("denm", [128, 4], F32)
    recm = sb("recm", [128, 4], F32)
    scm = sb("scm", [128, 4], F32)
    scmbf = sb("scmbf", [128, 4], BF16)
    ny1 = sb("ny1", [128, 32], F32)
    ny2 = sb("ny2", [128, 32], F32)
    msk1 = sb("msk1", [128, 32], F32)
    msk2 = sb("msk2", [128, 32], F32)
    w1 = sb("w1", [128, 32], F32)
    w2 = sb("w2", [128, 32], F32)
    w12 = sb("w12", [128, 64], BF16)
    m1ff = sb("m1ff", [128, 32], F32)
    m2ff = sb("m2ff", [128, 32], F32)
    sq = sb("sq", [128, 32], F32)  # scratch squares/products
    pr = sb("pr", [128, 32], F32)
    tgtf = sb("tgtf", [128, 32], F32)
    sel = sb("sel", [128, 32], F32)
    lt = sb("lt", [128, 32], F32)
    lnden = sb("lnden", [128, 32], F32)
    ce = sb("ce", [128, 32], F32)
    G = sb("G", [128, 12], F32)
    Gsb = sb("Gsb", [1, 12], F32)
    ones_t = sb("ones_t", [128, 1], F32)

    # double-buffered loop tiles
    d_ = [sb(f"d{i}", [128, 2 * M], BF16) for i in range(3)]
    a_ = [sb(f"a{i}", [128, 2 * M], BF16) for i in range(3)]
    b1_ = [sb(f"b1{i}", [128, 2 * M], BF16) for i in range(3)]
    b2_ = [sb(f"b2{i}", [128, 2 * M], BF16) for i in range(3)]
    ab1_ = [sb(f"ab1{i}", [128, 2 * M], BF16) for i in range(3)]
    ab2_ = [sb(f"ab2{i}", [128, 2 * M], BF16) for i in range(3)]

    mrow_w = sb("mrow_w", [2, M], F32)
    w12row = sb("w12row", [2, M], F32)
    arsb = sb("arsb", [2, M], F32)
    V = sb("V", [2, 2 * M], F32)
    t_ = sb("t_", [2, M], F32)
    t2_ = sb("t2_", [2, M], F32)
    sc2 = sb("sc2", [2, 8], F32)
    st_b2 = sb("st_b2", [2, M], F32)
    st_p2 = sb("st_p2", [2, M], F32)
    w12row_bf = sb("w12row_bf", [2, M], BF16)

    ps_a = ps("ps_a", [2, M])
    ps_b1 = ps("ps_b1", [2, M])
    ps_b2 = ps("ps_b2", [2, M])
    ps_p1 = ps("ps_p1", [2, M])
    ps_p2 = ps("ps_p2", [2, M])
    ps_g = ps("ps_g", [1, 12])

    dm = es.enter_context(nc.semaphore("dm"))
    sa = es.enter_context(nc.semaphore("sa"))  # ACT progress
    svp = es.enter_context(nc.semaphore("svp"))  # DVE phase-0 progress
    sv = es.enter_context(nc.semaphore("sv"))  # DVE loop progress
    sp = es.enter_context(nc.semaphore("sp"))  # PE progress
    sa2 = es.enter_context(nc.semaphore("sa2"))  # ACT phase-2 copies
    block = es.enter_context(nc.Block())

    @block.sync
    def _(sync):
        # 10 input DMAs (dm: 16 each -> 160 when all in)
        sync.dma_start(out=lgt[:], in_=lg[:].rearrange("(p f) c -> p (f c)", f=32)).then_inc(dm, 16)
        sync.dma_start(out=lgtm[:], in_=lgm[:].rearrange("(p q) c -> p (q c)", q=4)).then_inc(dm, 16)
        sync.dma_start(out=tgt[:], in_=tg[:].rearrange("(p f) -> p f", f=32)).then_inc(dm, 16)
        sync.dma_start(out=wt[:], in_=wf[:].rearrange("(p f) -> p f", f=32)).then_inc(dm, 16)
        sync.dma_start(out=m1t[:], in_=m1f[:].rearrange("(p f) -> p f", f=32)).then_inc(dm, 16)
        sync.dma_start(out=m2t[:], in_=m2f[:].rearrange("(p f) -> p f", f=32)).then_inc(dm, 16)
        sync.dma_start(out=wrow[:], in_=wm[:].broadcast_to([2, M])).then_inc(dm, 16)
        sync.dma_start(out=mrow[0:1, :], in_=m1m[:]).then_inc(dm, 16)
        sync.dma_start(out=mrow[1:2, :], in_=m2m[:]).then_inc(dm, 16)
        sync.dma_start(out=y1row[:], in_=m1m[:].broadcast_to([128, M])).then_inc(dm, 16)
        sync.dma_start(out=y2row[:], in_=m2m[:].broadcast_to([128, M])).then_inc(dm, 16)
        # scores-mine roundtrip: wait for scmbf (svp>=1)
        sync.wait_ge(svp, 1)
        sync.dma_start(out=scr[:].rearrange("a b -> (a b)"), in_=scmbf[:]).then_inc(dm, 16)
        sync.wait_ge(dm, 16 * 12)
        sync.dma_start(out=xrow[:], in_=scr[:].broadcast_to([128, M])).then_inc(dm, 16)
        # phase-2 row moves (partition 1 via DMA)
        sync.wait_ge(sa2, 4)
        sync.dma_start(out=V[1:2, 0:M], in_=st_b2[1:2, :]).then_inc(dm, 16)
        sync.dma_start(out=V[1:2, M : 2 * M], in_=st_p2[1:2, :]).then_inc(dm, 16)
        # outputs
        sync.wait_ge(sv, 200)
        sync.dma_start(out=out[0:12], in_=Gsb[:]).then_inc(dm, 16)
        sync.dma_start(out=out[12:28], in_=sc2[:]).then_inc(dm, 16)

    @block.scalar
    def _(scalar):
        scalar.wait_ge(dm, 16 * 11)  # all initial loads in
        scalar.activation(e[:], lgt[:], AF.Exp).then_inc(sa, 1)
        scalar.activation(em[:], lgtm[:], AF.Exp).then_inc(sa, 1)
        scalar.wait_ge(svp, 2)  # den ready
        scalar.activation(lnden[:], den[:], AF.Ln).then_inc(sa, 1)
        # loop: b1/b2 per j-set, double buffered
        scalar.wait_ge(svp, 3)  # ny1, ny2 ready
        for K in range(NB // 2):
            s = K % 3
            if K >= 3:
                scalar.wait_ge(sp, K - 1)
            k0, k1 = 2 * K, 2 * K + 1
            scalar.activation(b1_[s][:, 0:M], y1row[:], AF.Abs, bias=ny1[:, k0 : k0 + 1]).then_inc(sa, 1)
            scalar.activation(b1_[s][:, M : 2 * M], y1row[:], AF.Abs, bias=ny1[:, k1 : k1 + 1]).then_inc(sa, 1)
            scalar.activation(b2_[s][:, 0:M], y2row[:], AF.Abs, bias=ny2[:, k0 : k0 + 1]).then_inc(sa, 1)
            scalar.activation(b2_[s][:, M : 2 * M], y2row[:], AF.Abs, bias=ny2[:, k1 : k1 + 1]).then_inc(sa, 1)
        # phase 2 copies out of PSUM
        scalar.wait_ge(sp, NB // 2 + 1)
        scalar.activation(V[0:1, 0:M], ps_b1[0:1, :], AF.Copy).then_inc(sa2, 1)
        scalar.activation(V[0:1, M : 2 * M], ps_p1[0:1, :], AF.Copy).then_inc(sa2, 1)
        scalar.activation(st_b2[:], ps_b2[:], AF.Copy).then_inc(sa2, 1)
        scalar.activation(st_p2[:], ps_p2[:], AF.Copy).then_inc(sa2, 1)

    @block.vector
    def _(vector):
        # ---- phase 0 ----
        vector.memset(ones_t[:], 1.0)
        vector.drain()
        vector.wait_ge(sa, 2)
        vector.tensor_reduce(denm[:], em[:].rearrange("p (f c) -> p f c", c=3), AX.X, AO.add)
        vector.drain()
        vector.reciprocal(recm[:], denm[:])
        vector.drain()
        vector.tensor_tensor(scm[:], em[:, 0:12:3], recm[:], AO.mult)
        vector.drain()
        vector.tensor_copy(scmbf[:], scm[:])
        vector.drain().then_inc(svp, 1)
        vector.tensor_reduce(den[:], e[:].rearrange("p (f c) -> p f c", c=3), AX.X, AO.add)
        vector.drain()
        vector.reciprocal(rec[:], den[:])
        vector.drain()
        vector.tensor_tensor(sc[:], e[:, 0:96:3], rec[:], AO.mult)
        vector.drain()
        vector.tensor_copy(scbf[:], sc[:])
        vector.drain()
        vector.tensor_copy(sc_r[:], scbf[:])
        vector.drain().then_inc(svp, 1)
        vector.wait_ge(dm, 16 * 11)  # all initial loads in
        vector.tensor_scalar(ny1[:], m1t[:], -1.0, None, AO.mult)
        vector.drain()
        vector.tensor_scalar(ny2[:], m2t[:], -1.0, None, AO.mult)
        vector.drain().then_inc(svp, 1)
        vector.tensor_scalar(msk1[:], m1t[:], 0.0, None, AO.is_gt)
        vector.drain()
        vector.tensor_scalar(msk2[:], m2t[:], 0.0, None, AO.is_gt)
        vector.drain()
        vector.tensor_tensor(w1[:], wt[:], msk1[:], AO.mult)
        vector.drain()
        vector.tensor_tensor(w2[:], wt[:], msk2[:], AO.mult)
        vector.drain()
        vector.tensor_copy(w12[:, 0:64:2], w1[:])
        vector.drain()
        vector.tensor_copy(w12[:, 1:64:2], w2[:])
        vector.drain().then_inc(svp, 1)
        vector.tensor_copy(w1[:], w12[:, 0:64:2])
        vector.drain()
        vector.tensor_copy(w2[:], w12[:, 1:64:2])
        vector.drain()
        # CE: lt = logits[target]
        vector.tensor_copy(tgtf[:], tgt[:])
        vector.drain()
        vector.tensor_scalar(sel[:], tgtf[:], 0.0, None, AO.is_equal)
        vector.drain()
        vector.tensor_tensor(lt[:], lgt[:, 0:96:3], sel[:], AO.mult)
        vector.drain()
        vector.tensor_scalar(sel[:], tgtf[:], 1.0, None, AO.is_equal)
        vector.drain()
        vector.tensor_tensor(pr[:], lgt[:, 1:96:3], sel[:], AO.mult)
        vector.drain()
        vector.tensor_tensor(lt[:], lt[:], pr[:], AO.add)
        vector.drain()
        vector.tensor_scalar(sel[:], tgtf[:], 2.0, None, AO.is_equal)
        vector.drain()
        vector.tensor_tensor(pr[:], lgt[:, 2:96:3], sel[:], AO.mult)
        vector.drain()
        vector.tensor_tensor(lt[:], lt[:], pr[:], AO.add)
        vector.drain()
        vector.wait_ge(sa, 3)  # lnden
        vector.tensor_tensor(ce[:], lnden[:], lt[:], AO.subtract)
        vector.drain()
        vector.tensor_tensor(pr[:], wt[:], ce[:], AO.mult)
        vector.drain()
        # G columns: Sw S1 S2 CE m1 q1 my1 qy1 m2 q2 my2 qy2
        vector.tensor_reduce(G[:, 0:1], wt[:], AX.X, AO.add)
        vector.drain()
        vector.tensor_reduce(G[:, 1:2], w1[:], AX.X, AO.add)
        vector.drain()
        vector.tensor_reduce(G[:, 2:3], w2[:], AX.X, AO.add)
        vector.drain()
        vector.tensor_reduce(G[:, 3:4], pr[:], AX.X, AO.add)
        vector.drain()
        vector.tensor_copy(m1ff[:], m1t[:])
        vector.drain()
        vector.tensor_copy(m2ff[:], m2t[:])
        vector.drain()
        vector.tensor_tensor(pr[:], w1[:], sc_r[:], AO.mult)
        vector.drain()
        vector.tensor_reduce(G[:, 4:5], pr[:], AX.X, AO.add)
        vector.drain()
        vector.tensor_tensor(sq[:], sc_r[:], sc_r[:], AO.mult)
        vector.drain()
        vector.tensor_tensor(pr[:], w1[:], sq[:], AO.mult)
        vector.drain()
        vector.tensor_reduce(G[:, 5:6], pr[:], AX.X, AO.add)
        vector.drain()
        vector.tensor_tensor(pr[:], w1[:], m1ff[:], AO.mult)
        vector.drain()
        vector.tensor_reduce(G[:, 6:7], pr[:], AX.X, AO.add)
        vector.drain()
        vector.tensor_tensor(pr[:], m1ff[:], m1ff[:], AO.mult)
        vector.drain()
        vector.tensor_tensor(pr[:], w1[:], pr[:], AO.mult)
        vector.drain()
        vector.tensor_reduce(G[:, 7:8], pr[:], AX.X, AO.add)
        vector.drain()
        vector.tensor_tensor(pr[:], w2[:], sc_r[:], AO.mult)
        vector.drain()
        vector.tensor_reduce(G[:, 8:9], pr[:], AX.X, AO.add)
        vector.drain()
        vector.tensor_tensor(pr[:], w2[:], sq[:], AO.mult)
        vector.drain()
        vector.tensor_reduce(G[:, 9:10], pr[:], AX.X, AO.add)
        vector.drain()
        vector.tensor_tensor(pr[:], w2[:], m2ff[:], AO.mult)
        vector.drain()
        vector.tensor_reduce(G[:, 10:11], pr[:], AX.X, AO.add)
        vector.drain()
        vector.tensor_tensor(pr[:], m2ff[:], m2ff[:], AO.mult)
        vector.drain()
        vector.tensor_tensor(pr[:], w2[:], pr[:], AO.mult)
        vector.drain()
        vector.tensor_reduce(G[:, 11:12], pr[:], AX.X, AO.add)
        vector.drain().then_inc(svp, 1)
        # w12row for phase 2
        vector.tensor_scalar(mrow_w[:], mrow[:], 0.0, None, AO.is_gt)
        vector.drain()
        vector.tensor_tensor(w12row[:], wrow[:], mrow_w[:], AO.mult)
        vector.drain()
        vector.tensor_copy(w12row_bf[:], w12row[:])
        vector.drain()
        vector.tensor_copy(w12row[:], w12row_bf[:])
        vector.drain()
        # ---- phase 1 loop ----
        vector.wait_ge(dm, 16 * 13)  # xrow in
        for K in range(NB // 2):
            s = K % 3
            if K >= 3:
                vector.wait_ge(sp, K - 1)
            k0, k1 = 2 * K, 2 * K + 1
            vector.tensor_scalar(d_[s][:, 0:M], xrow[:], sc_r[:, k0 : k0 + 1], None, AO.subtract)
            vector.tensor_scalar(d_[s][:, M : 2 * M], xrow[:], sc_r[:, k1 : k1 + 1], None, AO.subtract)
            vector.drain()
            vector.tensor_scalar(
                a_[s][:].bitcast(U16), d_[s][:].bitcast(U16), 0x7FFF, None, AO.bitwise_and
            )
            vector.drain().then_inc(sv, 1)
            vector.wait_ge(sa, 3 + 4 * K + 2)
            vector.tensor_tensor(ab1_[s][:], a_[s][:], b1_[s][:], AO.mult)
            vector.wait_ge(sa, 3 + 4 * K + 4)
            vector.tensor_tensor(ab2_[s][:], a_[s][:], b2_[s][:], AO.mult)
            vector.drain().then_inc(sv, 1)
        # ---- phase 2 ----
        vector.wait_ge(sp, NB // 2 + 1)
        vector.tensor_copy(arsb[:], ps_a[:])
        vector.drain()
        vector.tensor_tensor(t_[:], arsb[:], w12row[:], AO.mult)
        vector.drain()
        vector.tensor_reduce(sc2[:, 0:1], t_[:], AX.X, AO.add)  # g_a
        vector.drain()
        vector.tensor_tensor(t2_[:], t_[:], arsb[:], AO.mult)
        vector.drain()
        vector.tensor_reduce(sc2[:, 1:2], t2_[:], AX.X, AO.add)  # T1aa
        vector.drain()
        vector.wait_ge(sa2, 4)
        vector.wait_ge(dm, 16 * 15)
        vector.tensor_tensor(t2_[:], V[:, 0:M], w12row[:], AO.mult)
        vector.drain()
        vector.tensor_reduce(sc2[:, 2:3], t2_[:], AX.X, AO.add)  # g_b
        vector.drain()
        vector.tensor_tensor(t2_[:], t2_[:], V[:, 0:M], AO.mult)
        vector.drain()
        vector.tensor_reduce(sc2[:, 3:4], t2_[:], AX.X, AO.add)  # T1bb
        vector.drain()
        vector.tensor_tensor(t2_[:], t_[:], V[:, 0:M], AO.mult)
        vector.drain()
        vector.tensor_reduce(sc2[:, 4:5], t2_[:], AX.X, AO.add)  # T1ab
        vector.drain()
        vector.tensor_tensor(t2_[:], V[:, M : 2 * M], w12row[:], AO.mult)
        vector.drain()
        vector.tensor_reduce(sc2[:, 5:6], t2_[:], AX.X, AO.add)  # S_ab
        vector.drain()
        vector.tensor_copy(Gsb[:], ps_g[:])
        vector.drain().then_inc(sv, 200)

    @block.tensor
    def _(tensor):
        tensor.wait_ge(svp, 5)  # G ready
        tensor.matmul(ps_g[:], ones_t[:], G[:], start=True, stop=True).then_inc(sp, 1)
        for K in range(NB // 2):
            s = K % 3
            st = K == 0
            last = K == NB // 2 - 1
            k0, k1 = 2 * K, 2 * K + 1
            lw0 = w12[:, 2 * k0 : 2 * k0 + 2]
            lw1 = w12[:, 2 * k1 : 2 * k1 + 2]
            tensor.wait_ge(sa, 3 + 4 * K + 4)
            tensor.matmul(ps_b1[:], lw0, b1_[s][:, 0:M], start=st, stop=False)
            tensor.matmul(ps_b1[:], lw1, b1_[s][:, M : 2 * M], start=False, stop=last)
            tensor.matmul(ps_b2[:], lw0, b2_[s][:, 0:M], start=st, stop=False)
            tensor.matmul(ps_b2[:], lw1, b2_[s][:, M : 2 * M], start=False, stop=last)
            tensor.wait_ge(sv, 2 * K + 2)
            tensor.matmul(ps_a[:], lw0, a_[s][:, 0:M], start=st, stop=False)
            tensor.matmul(ps_a[:], lw1, a_[s][:, M : 2 * M], start=False, stop=last)
            tensor.matmul(ps_p1[:], lw0, ab1_[s][:, 0:M], start=st, stop=False)
            tensor.matmul(ps_p1[:], lw1, ab1_[s][:, M : 2 * M], start=False, stop=last)
            tensor.matmul(ps_p2[:], lw0, ab2_[s][:, 0:M], start=st, stop=False)
            tensor.matmul(ps_p2[:], lw1, ab2_[s][:, M : 2 * M], start=False, stop=last).then_inc(sp, 1)

    return nc, es


_NC_CACHE = {}


def _build_callable():
    """Build the bass program once and wrap it in a persistently cached
    jitted shard_map callable. run_bass_kernel_spmd re-jits a fresh closure
    on every call, which re-traces/reloads over the device tunnel (~3x the
    per-call latency); caching the compiled callable makes a warm call cost
    a single transfer+execute+fetch round trip."""
    import jax
    from jax.sharding import Mesh, PartitionSpec
    from jax.experimental.shard_map import shard_map
    from concourse import bass2jax

    nc, es = _build_program()
    bass2jax.install_neuronx_cc_hook()

    partition_name = nc.partition_id_tensor.name if nc.partition_id_tensor else None
    in_names, out_names, out_avals, zero_outs = [], [], [], []
    for alloc in nc.m.functions[0].allocations:
        if not isinstance(alloc, mybir.MemoryLocationSet):
            continue
        name = alloc.memorylocations[0].name
        if alloc.kind == "ExternalInput":
            if name != partition_name:
                in_names.append(name)
        elif alloc.kind == "ExternalOutput":
            out_names.append(name)
            shape = tuple(alloc.tensor_shape)
            dtype = mybir.dt.np(alloc.dtype)
            out_avals.append(jax.core.ShapedArray(shape, dtype))
            zero_outs.append(np.zeros(shape, dtype))
    n_params = len(in_names)
    n_outs = len(out_avals)
    all_in_names = in_names + out_names + ([partition_name] if partition_name else [])
    donate = tuple(range(n_params, n_params + n_outs))

    def _body(*args):
        operands = list(args)
        if partition_name is not None:
            operands.append(bass2jax.partition_id_tensor())
        outs = bass2jax._bass_exec_p.bind(
            *operands,
            out_avals=tuple(out_avals),
            in_names=tuple(all_in_names),
            out_names=tuple(out_names),
            lowering_input_output_aliases=(),
            sim_require_finite=True,
            sim_require_nnan=True,
            nc=nc,
        )
        return tuple(outs)

    devices = jax.devices()[:NCORES]
    mesh = Mesh(np.asarray(devices), ("core",))
    in_specs = (PartitionSpec("core"),) * (n_params + n_outs)
    out_specs = (PartitionSpec("core"),) * n_outs
    sharded = jax.jit(
        shard_map(_body, mesh=mesh, in_specs=in_specs, out_specs=out_specs, check_rep=False),
        donate_argnums=donate,
        keep_unused=True,
    )
    return {
        "nc": (nc, es),
        "sharded": sharded,
        "in_names": in_names,
        "out_names": out_names,
        "zero_outs": zero_outs,
    }


def _concat_inputs(logits, target_i, weight, m1b, m2b):
    # Global (8*per_core_shape0, ...) arrays for shard_map's P("core") split.
    # Full-N params are tiled 8x; block params concatenate back to the full
    # array (e.g. concat of logits[c*M:(c+1)*M] over c IS logits).
    return {
        "lg": np.tile(logits, (NCORES, 1)),
        "lgm": logits,
        "tg": np.tile(target_i, NCORES),
        "wf": np.tile(weight, NCORES),
        "wm": weight.reshape(NCORES, M),
        "m1f": np.tile(m1b, NCORES),
        "m2f": np.tile(m2b, NCORES),
        "m1m": m1b.reshape(NCORES, M),
        "m2m": m2b.reshape(NCORES, M),
    }


def kernel(logits, target, weight, mass1, mass2):
    logits = np.ascontiguousarray(logits, dtype=np.float32)
    target_i = np.asarray(target).astype(np.int32)
    weight = np.ascontiguousarray(weight, dtype=np.float32)
    m1b = np.asarray(mass1, dtype=np.float32).astype(ml_dtypes.bfloat16)
    m2b = np.asarray(mass2, dtype=np.float32).astype(ml_dtypes.bfloat16)

    if "call" not in _NC_CACHE:
        _NC_CACHE["call"] = _build_callable()
    C = _NC_CACHE["call"]

    cin = _concat_inputs(logits, target_i, weight, m1b, m2b)
    args = [cin[name] for name in C["in_names"]]
    zeros = [np.zeros((NCORES * z.shape[0], *z.shape[1:]), z.dtype) for z in C["zero_outs"]]
    out_arrs = C["sharded"](*args, *zeros)
    outs = np.asarray(out_arrs[0]).reshape(NCORES, 32)
    return _combine(list(outs))


def _combine(outs):
    G = np.asarray(outs[0][0:12], dtype=np.float64)
    Sw, S1, S2, CEs = G[0], G[1], G[2], G[3]
    m1, q1, my1, qy1 = G[4], G[5], G[6], G[7]
    m2, q2, my2, qy2 = G[8], G[9], G[10], G[11]
    # per-core partials: sc2 [2,8] flattened at out[12:28]
    P = np.zeros((2, 8), dtype=np.float64)
    for o in outs:
        P += o[12:28].reshape(2, 8).astype(np.float64)
    ce_mean = CEs / max(Sw, EPS_W)

    def disco(row, Sr, m, q, my, qy):
        g_a, T1aa, g_b, T1bb, T1ab, S_ab = P[row, 0:6]
        s = 1.0 / max(Sr, EPS_W)
        dcov = s * s * S_ab - 2.0 * s**3 * T1ab + s**4 * g_a * g_b
        dvx = 2.0 * (s * q - (s * m) ** 2) - 2.0 * s**3 * T1aa + (s * s * g_a) ** 2
        dvy = 2.0 * (s * qy - (s * my) ** 2) - 2.0 * s**3 * T1bb + (s * s * g_b) ** 2
        ok = (dvx > EPS_VAR) and (dvy > EPS_VAR)
        if not ok:
            return 0.0
        return np.sqrt(np.abs(dcov) / np.sqrt(dvx * dvy))

    d1 = disco(0, S1, m1, q1, my1, qy1)
    d2 = disco(1, S2, m2, q2, my2, qy2)
    return np.float32(ce_mean + DISCO_LAMBDA * (d1 + d2))

